# revision 8
# baseline (speedup 1.0000x reference)
"""CQAttention (QANet context-query attention) Trainium2 Bass kernel.

Full-input contract: kernel(C, Q, cmask, qmask, w) -> (B, 4D, LC) f32.
Shards batch B=16 across 8 NeuronCores (2 examples/core), runs one SPMD
Bass/Tile program, gathers results.

Math (per example, d=512, Lc=2048, Lq=512):
  S = Cb@w1 [i] + Qb@w2 [j] + (Cb*w3)@Qb^T          (Lc, Lq)
  S1 = softmax_j(S), S2 = softmax_i(S)
  A = S1@Qb ; Bt = S1@S2^T@Cb
  out = concat([Cb, A, Cb*A, Cb*Bt], feat).T        (4d, Lc)

Kernel restructuring (all layouts kept "feature-on-partitions", which is
both the input layout of C/Q and the required output layout):
  - softmax_j drops the row bias, softmax_i drops the col bias, so with
    Qmod = w3*Q + w1 and Cmod = w3*C + w2:
      E2  = exp(S + r1)   = exp(C^T_chunks @ Qmod)   rows=i, cols=j
      E1T = exp(S^T + c2) = exp(Q^T_chunks @ Cmod)   rows=j, cols=i
    (max-subtraction skipped: |S+bias| <= ~8 for N(0,1) inputs, safe in
    fp32)
  - colsums via ones-vector matmuls; the axis-j softmax reciprocal row is
    replicated across partitions with a K=1 ones outer-product matmul and
    applied at the output multiplies; the axis-i softmax reciprocal is a
    per-partition tensor_scalar on T2 = S2raw^T@Cb.
  - A^T = Qb @ E1T, Bt^T = T2s^T @ E1T; final rows are elementwise
    products with resident fp32 C and the replicated reciprocal.
  - matmuls run in float32r (full PE rate at N=512). The BIR verifier
    requires f32r operands to be produced rounded, so every matmul input
    lives in a float32r tile written by DVE/ACT; exact fp32 copies of C
    are kept for the output elementwise stage.
"""

import numpy as np

import concourse.bass as bass
import concourse.tile as tile
from concourse import bacc, mybir
from concourse.bass_utils import run_bass_kernel_spmd
from concourse.masks import make_identity

B, D, LC, LQ = 16, 512, 2048, 512
NCORES = 8
BL = B // NCORES  # examples per core
KD = D // 128  # 4 d-chunks
KJ = LQ // 128  # 4 j-chunks
NI = LC // 512  # 4 i column-chunks
MI = LC // 128  # 16 i partition-chunks

F32 = mybir.dt.float32
F32R = mybir.dt.float32r
EXP = mybir.ActivationFunctionType.Exp
MUL = mybir.AluOpType.mult
ADD = mybir.AluOpType.add


def _example(tc, nc, consts, Cd, Qd, Od, b):
    """Emit one example's program. Cd/Qd/Od are DRAM APs for this core."""
    ident, identR, ones_col, ones_row, wsb = consts
    with tc.tile_pool(name=f"pers{b}", bufs=1) as pers, tc.tile_pool(
        name=f"psum{b}", space="PSUM", bufs=8
    ) as psum:
        Ct = pers.tile([128, KD, LC], F32)
        E1T = pers.tile([128, KJ, LC], F32R)
        Qbt = pers.tile([128, KJ, D], F32R)
        T2s = pers.tile([128, KJ, D], F32R)
        rec1rep = pers.tile([128, NI, 512], F32)
        rec1flat = rec1rep.rearrange("p a c -> p (a c)")

        nc.sync.dma_start(out=Ct, in_=Cd[b].rearrange("(k p) i -> p k i", p=128))
        # out rows 0..D-1 are exactly C[b]
        nc.sync.dma_start(
            out=Od[b, 0:D, :].rearrange("(k p) i -> p k i", p=128), in_=Ct
        )

        with tc.tile_pool(name=f"mid{b}", bufs=1) as mid:
            QtR = mid.tile([128, KD, LQ], F32R)
            Qmod = mid.tile([128, KD, LQ], F32R)
            CtR = mid.tile([128, KD, LC], F32R)
            Cmod = mid.tile([128, KD, LC], F32R)

            with tc.tile_pool(name=f"qload{b}", bufs=1) as qload:
                Qt = qload.tile([128, KD, LQ], F32)
                nc.sync.dma_start(
                    out=Qt, in_=Qd[b].rearrange("(k p) j -> p k j", p=128)
                )
                for k in range(KD):
                    # wsb cols: 0-3 w1, 4-7 w2, 8-11 w3
                    nc.vector.tensor_copy(QtR[:, k, :], Qt[:, k, :])
                    nc.vector.tensor_scalar(
                        out=Qmod[:, k, :], in0=Qt[:, k, :],
                        scalar1=wsb[:, 8 + k : 9 + k], scalar2=wsb[:, k : k + 1],
                        op0=MUL, op1=ADD,
                    )
                    nc.vector.tensor_scalar(
                        out=Cmod[:, k, :], in0=Ct[:, k, :],
                        scalar1=wsb[:, 8 + k : 9 + k], scalar2=wsb[:, 4 + k : 5 + k],
                        op0=MUL, op1=ADD,
                    )
                    nc.vector.tensor_copy(CtR[:, k, :], Ct[:, k, :])
                # Qbt[j, c, d] = Q[b]^T, via PE transposes of 128x128 blocks
                for c in range(KJ):
                    qps = psum.tile([128, D], F32, tag="ps", name=f"qps{b}_{c}")
                    for a in range(KD):
                        nc.tensor.transpose(
                            qps[:, a * 128 : (a + 1) * 128],
                            Qt[:, a, c * 128 : (c + 1) * 128],
                            ident,
                        )
                    nc.vector.tensor_copy(Qbt[:, c, :], qps)

            with tc.tile_pool(name=f"stream{b}", bufs=2) as stream:
                # ---- streamed: E2 row-chunks -> T2 accumulation + ssum2
                t2ps = [
                    psum.tile([128, D], F32, tag="ps", name=f"t2ps{b}_{m}")
                    for m in range(KJ)
                ]
                ssps = psum.tile([1, LQ], F32, tag="ps")
                for ki in range(MI):
                    isl = slice(ki * 128, (ki + 1) * 128)
                    cbt_ps = psum.tile([128, D], F32R, tag="ps", name=f"cps{b}_{ki}")
                    for kd in range(KD):
                        nc.tensor.transpose(
                            cbt_ps[:, kd * 128 : (kd + 1) * 128],
                            CtR[:, kd, isl],
                            identR,
                        )
                    cbt_sb = stream.tile(
                        [128, D], F32R, tag="cbt", name=f"cbt{b}_{ki}"
                    )
                    nc.vector.tensor_copy(cbt_sb, cbt_ps)

                    e2ps = psum.tile([128, LQ], F32, tag="ps", name=f"e2ps{b}_{ki}")
                    for kd in range(KD):
                        nc.tensor.matmul(
                            e2ps, CtR[:, kd, isl], Qmod[:, kd, :],
                            start=(kd == 0), stop=(kd == KD - 1),
                        )
                    e2sb = stream.tile(
                        [128, LQ], F32R, tag="e2", name=f"e2sb{b}_{ki}"
                    )
                    nc.scalar.activation(e2sb, e2ps, EXP)

                    nc.tensor.matmul(
                        ssps, ones_col, e2sb,
                        start=(ki == 0), stop=(ki == MI - 1),
                    )
                    for mj in range(KJ):
                        nc.tensor.matmul(
                            t2ps[mj], e2sb[:, mj * 128 : (mj + 1) * 128], cbt_sb,
                            start=(ki == 0), stop=(ki == MI - 1),
                        )

                rec2row = stream.tile([1, LQ], F32, tag="rec2row")
                nc.vector.reciprocal(rec2row, ssps)
                rc_ps = psum.tile([128, KJ], F32, tag="ps")
                for jm in range(KJ):
                    nc.tensor.transpose(
                        rc_ps[:, jm : jm + 1],
                        rec2row[:, jm * 128 : (jm + 1) * 128],
                        ident[:1, :1],
                    )
                rec2col = stream.tile([128, KJ], F32, tag="rec2col")
                nc.vector.tensor_copy(rec2col, rc_ps)
                for mj in range(KJ):
                    nc.vector.tensor_scalar(
                        out=T2s[:, mj, :], in0=t2ps[mj],
                        scalar1=rec2col[:, mj : mj + 1], scalar2=None, op0=MUL,
                    )

                # ---- E1T = exp(Q^T_chunks @ Cmod)
                for mj in range(KJ):
                    for ni in range(NI):
                        nsl = slice(ni * 512, (ni + 1) * 512)
                        e1ps = psum.tile(
                            [128, 512], F32, tag="ps", name=f"e1ps{b}_{mj}_{ni}"
                        )
                        for kd in range(KD):
                            nc.tensor.matmul(
                                e1ps,
                                QtR[:, kd, mj * 128 : (mj + 1) * 128],
                                Cmod[:, kd, nsl],
                                start=(kd == 0), stop=(kd == KD - 1),
                            )
                        nc.scalar.activation(E1T[:, mj, nsl], e1ps, EXP)

                # ---- colsum over j of E1T -> replicate -> reciprocal
                for ni in range(NI):
                    nsl = slice(ni * 512, (ni + 1) * 512)
                    csps = psum.tile([1, 512], F32, tag="ps", name=f"csps{b}_{ni}")
                    for kj in range(KJ):
                        nc.tensor.matmul(
                            csps, ones_col, E1T[:, kj, nsl],
                            start=(kj == 0), stop=(kj == KJ - 1),
                        )
                    csrow = stream.tile([1, 512], F32R, tag="csrow", name=f"cs{b}_{ni}")
                    nc.vector.tensor_copy(csrow, csps)
                    repps = psum.tile(
                        [128, 512], F32, tag="ps", name=f"repps{b}_{ni}"
                    )
                    nc.tensor.matmul(repps, ones_row, csrow, start=True, stop=True)
                    nc.vector.reciprocal(rec1rep[:, ni, :], repps)

        # ---- outputs: A^T, C*A^T, C*Bt^T  (rows d, cols i)
        with tc.tile_pool(name=f"ost{b}", bufs=1) as ost:
            for md in range(4):
                msl = slice(md * 128, (md + 1) * 128)
                crt = ost.tile([128, LC], F32, tag="crt", bufs=2, name=f"crt{b}_{md}")
                nc.vector.tensor_mul(crt, Ct[:, md, :], rec1flat)
                out2t = ost.tile([128, LC], F32, tag="o2", bufs=2, name=f"o2_{b}_{md}")
                out3t = ost.tile([128, LC], F32, tag="o3", bufs=2, name=f"o3_{b}_{md}")
                out4t = ost.tile([128, LC], F32, tag="o4", bufs=2, name=f"o4_{b}_{md}")
                for ni in range(NI):
                    nsl = slice(ni * 512, (ni + 1) * 512)
                    aps = psum.tile(
                        [128, 512], F32, tag="ps", name=f"aps{b}_{md}_{ni}"
                    )
                    for kj in range(KJ):
                        nc.tensor.matmul(
                            aps, Qbt[:, kj, msl], E1T[:, kj, nsl],
                            start=(kj == 0), stop=(kj == KJ - 1),
                        )
                    nc.vector.tensor_mul(out2t[:, nsl], aps, rec1rep[:, ni, :])
                    nc.vector.tensor_mul(out3t[:, nsl], aps, crt[:, nsl])
                for ni in range(NI):
                    nsl = slice(ni * 512, (ni + 1) * 512)
                    bps = psum.tile(
                        [128, 512], F32, tag="ps", name=f"bps{b}_{md}_{ni}"
                    )
                    for kj in range(KJ):
                        nc.tensor.matmul(
                            bps, T2s[:, kj, msl], E1T[:, kj, nsl],
                            start=(kj == 0), stop=(kj == KJ - 1),
                        )
                    nc.vector.tensor_mul(out4t[:, nsl], bps, crt[:, nsl])
                nc.sync.dma_start(
                    out=Od[b, D + md * 128 : D + (md + 1) * 128, :], in_=out2t
                )
                nc.sync.dma_start(
                    out=Od[b, 2 * D + md * 128 : 2 * D + (md + 1) * 128, :], in_=out3t
                )
                nc.sync.dma_start(
                    out=Od[b, 3 * D + md * 128 : 3 * D + (md + 1) * 128, :], in_=out4t
                )


def build(bl=BL, num_devices=NCORES, enable_asserts=False):
    nc = bacc.Bacc(
        "TRN2",
        target_bir_lowering=False,
        debug=False,
        enable_asserts=enable_asserts,
        num_devices=num_devices,
    )
    Cd = nc.dram_tensor("C", (bl, D, LC), F32, kind="ExternalInput").ap()
    Qd = nc.dram_tensor("Q", (bl, D, LQ), F32, kind="ExternalInput").ap()
    wd = nc.dram_tensor("w", (3 * D,), F32, kind="ExternalInput").ap()
    Od = nc.dram_tensor("out", (bl, 4 * D, LC), F32, kind="ExternalOutput").ap()

    with tile.TileContext(nc) as tc:
        with tc.tile_pool(name="const", bufs=1) as constp:
            ident = constp.tile([128, 128], F32)
            make_identity(nc, ident)
            identR = constp.tile([128, 128], F32R)
            nc.vector.tensor_copy(identR, ident)
            ones_col_f = constp.tile([128, 1], F32)
            nc.vector.memset(ones_col_f, 1.0)
            ones_col = constp.tile([128, 1], F32R)
            nc.vector.tensor_copy(ones_col, ones_col_f)
            ones_row_f = constp.tile([1, 128], F32)
            nc.vector.memset(ones_row_f, 1.0)
            ones_row = constp.tile([1, 128], F32R)
            nc.vector.tensor_copy(ones_row, ones_row_f)
            wsb = constp.tile([128, 12], F32)
            nc.sync.dma_start(out=wsb, in_=wd.rearrange("(c p) -> p c", p=128))
            consts = (ident, identR, ones_col, ones_row, wsb)
            for b in range(bl):
                _example(tc, nc, consts, Cd, Qd, Od, b)
    nc.compile()
    return nc


_NC = None


def kernel(C, Q, cmask, qmask, w):
    global _NC
    C = np.ascontiguousarray(np.asarray(C, dtype=np.float32))
    Q = np.ascontiguousarray(np.asarray(Q, dtype=np.float32))
    w = np.ascontiguousarray(np.asarray(w, dtype=np.float32))
    # masks are all-ones per the problem spec; softmax masking is a no-op
    if _NC is None:
        _NC = build()
    in_maps = [
        {
            "C": np.ascontiguousarray(C[i * BL : (i + 1) * BL]),
            "Q": np.ascontiguousarray(Q[i * BL : (i + 1) * BL]),
            "w": w,
        }
        for i in range(NCORES)
    ]
    res = run_bass_kernel_spmd(_NC, in_maps, core_ids=list(range(NCORES)))
    return np.concatenate([res.results[i]["out"] for i in range(NCORES)], axis=0)
